# revision 2
# baseline (speedup 1.0000x reference)
"""Multi-head attention (B=2, S=2048, D=1024, H=16, d_k=d_v=64) on 8 TRN2 cores.

Sharding: core c handles batch b = c // 4 and heads (c % 4) * 4 .. + 4.
Per core the [4, S, S] mask slice is read and the [4, S, S] attention
probability slice is written — the dominant (memory-bound) traffic.

Device pipeline per core:
  phase 1: QT/KT = (Wq|Wk)^T X^T and V = X Wv  from a host-pretransposed,
           bias-augmented X^T (biases fold in via a ones row).  fp32r.
  phase 2: per (head, 512-q super-tile): scores (PE, fp32r) -> exp (ACT)
           -> *mask + rowsum (DVE scalar_tensor_tensor, int32 mask)
           -> *1/rowsum in place (DVE) -> DMA attn out
           -> PE-transpose attn -> PT (bf16) -> ctxT += V^T @ PT (PE)
  phase 3: out_partial = ctxT^T @ Wo_rows (PE) -> DMA.
Host sums the 4 per-core out partials of each batch and adds bo.
"""
import numpy as np

B, S, D_MODEL, H, D_K, D_V = 2, 2048, 1024, 16, 64, 64
H_CORE = 4              # heads per core
N_CORES = 8
M_AUG = 1152            # 1024 X rows + 1 ones row + 127 zero rows
M_CHUNKS = M_AUG // 128
W_COLS = 3 * H_CORE * D_K   # 768: [Q 256 | K 256 | V 256]

_PROGRAM = None


def _build_program():
    from contextlib import ExitStack

    import concourse.bacc as bacc
    import concourse.mybir as mybir
    import concourse.tile as tile
    from concourse.masks import make_identity

    f32 = mybir.dt.float32
    f32r = mybir.dt.float32r
    bf16 = mybir.dt.bfloat16
    i32 = mybir.dt.int32
    MULT = mybir.AluOpType.mult
    EXP = mybir.ActivationFunctionType.Exp

    nc = bacc.Bacc("TRN2", target_bir_lowering=False)

    xt_d = nc.dram_tensor("xt", [M_AUG, S], f32, kind="ExternalInput")
    w_d = nc.dram_tensor("w", [M_AUG, W_COLS], f32, kind="ExternalInput")
    wo_d = nc.dram_tensor("wo", [H_CORE * D_V, D_MODEL], f32, kind="ExternalInput")
    mask_d = nc.dram_tensor("mask", [H_CORE, S, S], i32, kind="ExternalInput")
    attn_d = nc.dram_tensor("attn_o", [H_CORE, S, S], f32, kind="ExternalOutput")
    out_d = nc.dram_tensor("out_o", [S, D_MODEL], f32, kind="ExternalOutput")

    with tile.TileContext(nc) as tc, ExitStack() as ctx:
        static = ctx.enter_context(tc.tile_pool(name="static", bufs=1))

        ident = static.tile([128, 128], f32)
        make_identity(nc, ident[:])

        # persistent activation/weight tiles
        qt_t = static.tile([128, 2, S], f32r)    # [d, head-pair, s]
        kt_t = static.tile([128, 2, S], f32r)
        v_t = static.tile([128, 16, H_CORE * D_V], bf16)   # [k_in, k_chunk, h*64+j]
        wo_t = static.tile([128, 2, D_MODEL], f32r)        # [j_in, j_chunk, n]

        # ---------------- phase 1: projections ----------------
        with tc.tile_pool(name="ph1", bufs=1) as ph1, \
             tc.tile_pool(name="ph1in", bufs=2) as ph1in, \
             tc.tile_pool(name="ph1psum", bufs=2, space="PSUM") as pp1:
            with nc.named_scope("phase1"):
                xt_r = ph1.tile([128, M_CHUNKS, S], f32r)
                w_r = ph1.tile([128, M_CHUNKS, W_COLS], f32r)
                for mi in range(M_CHUNKS):
                    xc = ph1in.tile([128, S], f32, tag="xc")
                    nc.sync.dma_start(xc[:], xt_d.ap()[mi * 128:(mi + 1) * 128, :])
                    nc.vector.tensor_copy(xt_r[:, mi, :], xc[:])
                    wc = ph1in.tile([128, W_COLS], f32, tag="wc")
                    nc.sync.dma_start(wc[:], w_d.ap()[mi * 128:(mi + 1) * 128, :])
                    nc.vector.tensor_copy(w_r[:, mi, :], wc[:])
                woc = ph1.tile([128, 2, D_MODEL], f32)
                nc.sync.dma_start(
                    woc[:], wo_d.ap().rearrange("(c p) n -> p c n", p=128))
                nc.vector.tensor_copy(wo_t[:], woc[:])

                # QT / KT: [d, s] = W[:, cols]^T @ XT
                for which, dst in ((0, qt_t), (1, kt_t)):
                    for pt in range(2):        # head pair -> 128 d-columns
                        c0 = which * 256 + pt * 128
                        for sc in range(4):    # s in chunks of 512
                            ps = pp1.tile([128, 512], f32, tag="projp", name="projp")
                            for mi in range(M_CHUNKS):
                                nc.tensor.matmul(
                                    ps[:],
                                    w_r[:, mi, c0:c0 + 128],
                                    xt_r[:, mi, sc * 512:(sc + 1) * 512],
                                    start=(mi == 0), stop=(mi == M_CHUNKS - 1))
                            nc.scalar.copy(dst[:, pt, sc * 512:(sc + 1) * 512], ps[:])

                # V: [s, h*64+j] = XT^T @ W[:, 512:768]
                for st in range(16):
                    ps = pp1.tile([128, 256], f32, tag="projv", name="projv")
                    for mi in range(M_CHUNKS):
                        nc.tensor.matmul(
                            ps[:],
                            xt_r[:, mi, st * 128:(st + 1) * 128],
                            w_r[:, mi, 512:768],
                            start=(mi == 0), stop=(mi == M_CHUNKS - 1))
                    nc.scalar.copy(v_t[:, st, :], ps[:])

        # ---------------- phase 2: attention ----------------
        with tc.tile_pool(name="ctxp", bufs=1) as ctxp:
            ctxT_t = ctxp.tile([128, 2, S], f32r)   # [h%2*64+j, h//2, q]

            with tc.tile_pool(name="maskp", bufs=2) as maskp, \
                 tc.tile_pool(name="ep", bufs=3) as ep, \
                 tc.tile_pool(name="ptp", bufs=2) as ptp, \
                 tc.tile_pool(name="rsp", bufs=8) as rsp, \
                 tc.tile_pool(name="spsum", bufs=2, space="PSUM") as spsum, \
                 tc.tile_pool(name="ptpsum", bufs=2, space="PSUM") as ptpsum, \
                 tc.tile_pool(name="ctxpsum", bufs=2, space="PSUM") as ctxpsum, \
                 nc.named_scope("phase2"):
                for h in range(H_CORE):
                    hp = h // 2
                    dlo = (h % 2) * 64
                    for sq in range(4):            # 512-wide q super-tile
                        ptsb = ptp.tile([128, 16, 512], bf16, tag="ptsb", name="ptsb")
                        for qp in range(2):        # pair of 128-q tiles
                            q0 = sq * 512 + qp * 256
                            mt = maskp.tile([128, 2, S], i32, tag="mt", name="mt")
                            nc.sync.dma_start(
                                mt[:],
                                mask_d.ap()[h, q0:q0 + 256, :]
                                .rearrange("(t p) k -> p t k", p=128))
                            et = ep.tile([128, 2, S], f32, tag="et", name="et")
                            for qi in range(2):
                                qt_idx = qp * 2 + qi
                                qrow = q0 + qi * 128
                                for sh in range(2):
                                    sp = spsum.tile([128, 1024], f32, tag="sps",
                                                    name="sps")
                                    for kc4 in range(2):
                                        k0 = sh * 1024 + kc4 * 512
                                        nc.tensor.matmul(
                                            sp[:, kc4 * 512:(kc4 + 1) * 512],
                                            qt_t[dlo:dlo + 64, hp, qrow:qrow + 128],
                                            kt_t[dlo:dlo + 64, hp, k0:k0 + 512],
                                            start=True, stop=True)
                                    nc.scalar.activation(
                                        et[:, qi, sh * 1024:(sh + 1) * 1024], sp[:],
                                        EXP)
                                rs = rsp.tile([128, 1], f32, tag="rs", name="rs")
                                nc.vector.scalar_tensor_tensor(
                                    et[:, qi, :], et[:, qi, :], 1.0, mt[:, qi, :],
                                    MULT, MULT, accum_out=rs[:])
                                rec = rsp.tile([128, 1], f32, tag="rec", name="rec")
                                nc.vector.reciprocal(rec[:], rs[:])
                                nc.vector.tensor_scalar(
                                    et[:, qi, :], et[:, qi, :], rec[:], None, MULT)
                                for g in range(4):
                                    pps = ptpsum.tile([128, 512], f32, tag="pps",
                                                      name="pps")
                                    for j in range(4):
                                        kc = g * 4 + j
                                        nc.tensor.transpose(
                                            pps[:, j * 128:(j + 1) * 128],
                                            et[:, qi, kc * 128:(kc + 1) * 128],
                                            ident[:])
                                    dst = ptsb[:, g * 4:(g + 1) * 4,
                                               qt_idx * 128:(qt_idx + 1) * 128]
                                    if (qt_idx + g) % 2 == 0:
                                        nc.scalar.copy(dst, pps[:])
                                    else:
                                        nc.vector.tensor_copy(dst, pps[:])
                            nc.sync.dma_start(
                                attn_d.ap()[h, q0:q0 + 256, :]
                                .rearrange("(t p) k -> p t k", p=128),
                                et[:])
                        cps = ctxpsum.tile([64, 512], f32, tag="cps", name="cps")
                        for kc in range(16):
                            nc.tensor.matmul(
                                cps[:],
                                v_t[:, kc, h * 64:(h + 1) * 64],
                                ptsb[:, kc, :],
                                start=(kc == 0), stop=(kc == 15))
                        nc.scalar.copy(
                            ctxT_t[dlo:dlo + 64, hp, sq * 512:(sq + 1) * 512], cps[:])

            # ---------------- phase 3: output projection ----------------
            with tc.tile_pool(name="outp", bufs=2) as outp, \
                 tc.tile_pool(name="opsum", bufs=4, space="PSUM") as opsum:
                with nc.named_scope("phase3"):
                    for sp2 in range(8):
                        ot = outp.tile([128, 2, D_MODEL], f32, tag="ot", name="ot")
                        for si in range(2):
                            st = sp2 * 2 + si
                            for nh in range(2):
                                ps = opsum.tile([128, 512], f32, tag="ops",
                                                name="ops")
                                for jc in range(2):
                                    nc.tensor.matmul(
                                        ps[:],
                                        ctxT_t[:, jc, st * 128:(st + 1) * 128],
                                        wo_t[:, jc, nh * 512:(nh + 1) * 512],
                                        start=(jc == 0), stop=(jc == 1))
                                nc.vector.tensor_copy(
                                    ot[:, si, nh * 512:(nh + 1) * 512], ps[:])
                        nc.sync.dma_start(
                            out_d.ap()[sp2 * 256:(sp2 + 1) * 256, :]
                            .rearrange("(t p) n -> p t n", p=128),
                            ot[:])

    nc.compile()
    return nc


def _get_program():
    global _PROGRAM
    if _PROGRAM is None:
        _PROGRAM = _build_program()
    return _PROGRAM


def kernel(X, Wq, bq, Wk, bk, Wv, bv, Wo, bo, mask):
    from concourse import bass_utils

    X = np.asarray(X, dtype=np.float32)
    Wq = np.asarray(Wq, dtype=np.float32)
    bq = np.asarray(bq, dtype=np.float32)
    Wk = np.asarray(Wk, dtype=np.float32)
    bk = np.asarray(bk, dtype=np.float32)
    Wv = np.asarray(Wv, dtype=np.float32)
    bv = np.asarray(bv, dtype=np.float32)
    Wo = np.asarray(Wo, dtype=np.float32)
    bo = np.asarray(bo, dtype=np.float32)
    mask = np.ascontiguousarray(np.asarray(mask, dtype=np.int32))

    scale = np.float32(1.0 / np.sqrt(D_K))

    nc = _get_program()

    # host-side shard prep
    xt_aug = []
    for b in range(B):
        xa = np.zeros((M_AUG, S), dtype=np.float32)
        xa[:D_MODEL] = X[b].T
        xa[D_MODEL] = 1.0
        xt_aug.append(xa)

    in_maps = []
    for c in range(N_CORES):
        b = c // 4
        hsl = slice((c % 4) * H_CORE * D_K, (c % 4 + 1) * H_CORE * D_K)
        w_aug = np.zeros((M_AUG, W_COLS), dtype=np.float32)
        w_aug[:D_MODEL, 0:256] = Wq[:, hsl]
        w_aug[D_MODEL, 0:256] = bq[hsl]
        w_aug[:D_MODEL, 256:512] = Wk[:, hsl] * scale
        w_aug[D_MODEL, 256:512] = bk[hsl] * scale
        w_aug[:D_MODEL, 512:768] = Wv[:, hsl]
        w_aug[D_MODEL, 512:768] = bv[hsl]
        in_maps.append({
            "xt": xt_aug[b],
            "w": w_aug,
            "wo": np.ascontiguousarray(Wo[hsl, :]),
            "mask": np.ascontiguousarray(
                mask[b, (c % 4) * H_CORE:(c % 4 + 1) * H_CORE]),
        })

    res = bass_utils.run_bass_kernel_spmd(
        nc, in_maps, core_ids=list(range(N_CORES)))

    attn = np.empty((B, H, S, S), dtype=np.float32)
    out = np.empty((B, S, D_MODEL), dtype=np.float32)
    for b in range(B):
        acc = None
        for c in range(4 * b, 4 * b + 4):
            r = res.results[c]
            attn[b, (c % 4) * H_CORE:(c % 4 + 1) * H_CORE] = r["attn_o"]
            acc = r["out_o"] if acc is None else acc + r["out_o"]
        out[b] = acc + bo
    return out, attn
